# revision 13
# baseline (speedup 1.0000x reference)
"""Self-contained 2-layer GAT kernel for Trainium2, 8-core SPMD.

Strategy (per the sharding hint): edges are sharded by destination node
across the 8 cores (each core owns a contiguous dst slice; edges sorted by
dst on host). Layer weights are replicated. The node-parallel dense work
(x@W plus the per-node attention halves) is replicated on every core so the
edge phase needs no cross-core halo exchange; the layer-1 -> layer-2
activation reshuffle is pure data movement done on host between the two
launches (all math stays on device).

Per layer, per core:
  node phase:  [h | aS | aD] = xT_chunk.T @ [W | W@Asrc | W@Adst]  (PE)
               -> htab (192-float rows: h|aS|pad) and adtab (64-float rows:
               aD|pad) in DRAM
  edge phase (per 128-dst tile, edges padded to 128-chunks):
      dma_gather h|aS rows by src, aD rows by dst    (int16 indices, so
          tables are split in halves; each tile's edges are bucketed lo/hi
          by src on host; dst reads hit one half per core, the other half's
          gather is pointed at a dedicated zero row)
      alpha = lrelu(aS+aDlo+aDhi); ex = exp(alpha)             (DVE+ACT)
      Ind[e,d] = (iota == dstloc[e]) one-hot                   (DVE)
      PSUM accum over chunks: [out|denom] += Ind.T @ [ex*h | ex]   (PE)
      epilogue: relu(out/denom + b) -> output rows.
"""

import sys
import numpy as np

sys.path.insert(0, "/opt/trn_rl_repo")

import concourse.bacc as bacc
import concourse.mybir as mybir
from concourse.bass_utils import run_bass_kernel_spmd
from concourse.tile import TileContext

f32 = mybir.dt.float32
bf16 = mybir.dt.bfloat16
i16 = mybir.dt.int16
i32 = mybir.dt.int32

P = 128
H = 4
C = 32
F = 128          # feature width (= H*C)
FA = F + H       # h | aS used columns
FW = F + 2 * H   # node-phase matmul width: h | aS | aD
HROW = 192       # htab row floats (768B, 256B multiple for dma_gather)
AROW = 64        # adtab row floats (256B)

N_CORES = 8
N_NODES = 50000
N_EDGES = 800000

EDGE_DT = f32    # dtype for the edge-phase matmul operands

import os
_SKIP = set(os.environ.get("GAT_SKIP", "").split(","))  # perf-bisect flags


def _make_plan(src, dst, N, n_cores):
    npad = ((N + P * n_cores - 1) // (P * n_cores)) * (P * n_cores)
    npc = npad // n_cores
    NT = npc // P
    NCH = npad // P
    NLO = npad // 2                # nodes in the lo half
    L = NLO + P                    # half-table rows (incl. one zero chunk)
    NROW = 2 * L                   # total table rows
    assert L <= 32767 and NLO % P == 0
    ZLO = NLO                      # half-local zero-row id (both halves)

    def rowid_half(v):
        """node id -> (half, half-local row id)"""
        return (v >= NLO).astype(np.int64), np.where(v >= NLO, v - NLO, v)

    tile_of = dst // P
    src_hi = (src >= NLO).astype(np.int64)

    cnt = np.zeros((NCH, 2), np.int64)
    np.add.at(cnt, (tile_of, src_hi), 1)
    cnt_ct = cnt.reshape(n_cores, NT, 2)
    Klo = np.maximum(1, np.ceil(cnt_ct[:, :, 0].max(axis=0) / P).astype(np.int64))
    Khi = np.ceil(cnt_ct[:, :, 1].max(axis=0) / P).astype(np.int64)
    K = Klo + Khi
    TOTCH = int(K.sum())
    CO = np.concatenate([[0], np.cumsum(K)])[:-1]

    okey = tile_of * 2 + src_hi
    order = np.argsort(okey, kind="stable")
    s_src = src[order]
    s_dst = dst[order]
    s_key = okey[order]
    starts = np.searchsorted(s_key, np.arange(NCH * 2))
    ends = np.searchsorted(s_key, np.arange(NCH * 2) + 1)

    gsrci = np.zeros((n_cores, 16, 8 * TOTCH), np.int16)
    gdlo = np.full((n_cores, 16, 8 * TOTCH), ZLO, np.int16)
    gdhi = np.full((n_cores, 16, 8 * TOTCH), ZLO, np.int16)
    gloc = np.full((n_cores, P, TOTCH), float(P), np.float32)

    for c in range(n_cores):
        for t in range(NT):
            g = c * NT + t
            co8 = 8 * CO[t]
            for half, koff, nk in ((0, 0, Klo[t]), (1, Klo[t], K[t] - Klo[t])):
                if nk == 0:
                    continue
                e0, e1 = starts[2 * g + half], ends[2 * g + half]
                n = e1 - e0
                npadn = int(nk) * P
                sv = np.zeros(npadn, np.int64)
                lv = np.full(npadn, P, np.int64)
                dhalf = np.zeros(npadn, np.int64)
                dloc = np.full(npadn, ZLO, np.int64)
                if n:
                    ev = s_src[e0:e1]
                    sv[:n] = np.where(ev >= NLO, ev - NLO, ev)
                    dh, dr = rowid_half(s_dst[e0:e1])
                    dhalf[:n] = dh
                    dloc[:n] = dr
                    lv[:n] = s_dst[e0:e1] % P
                j = np.arange(npadn)
                cc = co8 + 8 * koff + j // 16
                rr = j % 16
                gsrci[c, rr, cc] = sv
                gdlo[c, rr, cc] = np.where(dhalf == 0, dloc, ZLO)
                gdhi[c, rr, cc] = np.where(dhalf == 1, dloc, ZLO)
                kk = CO[t] + koff + j // P
                pp = j % P
                gloc[c, pp, kk] = lv

    gsrci = np.tile(gsrci, (1, 8, 1))
    gdlo = np.tile(gdlo, (1, 8, 1))
    gdhi = np.tile(gdhi, (1, 8, 1))

    return dict(
        n_cores=n_cores, N=N, npad=npad, npc=npc, NT=NT, NCH=NCH,
        NLO=NLO, L=L, NROW=NROW,
        K=[int(k) for k in K], Klo=[int(k) for k in Klo],
        TOTCH=TOTCH, CO=[int(o) for o in CO],
        gsrci=gsrci, gdlo=gdlo, gdhi=gdhi, gloc=gloc,
    )


def _layer_inputs(plan, x, W, a_src, a_dst, b):
    npad, NCH = plan["npad"], plan["NCH"]
    xp = np.zeros((npad, F), dtype=np.float32)
    xp[: x.shape[0]] = x
    xt = np.ascontiguousarray(xp.reshape(NCH, P, F).transpose(0, 2, 1))
    Ablk_s = np.zeros((F, H), dtype=np.float32)
    Ablk_d = np.zeros((F, H), dtype=np.float32)
    for h in range(H):
        Ablk_s[h * C:(h + 1) * C, h] = a_src[h]
        Ablk_d[h * C:(h + 1) * C, h] = a_dst[h]
    W = np.asarray(W, dtype=np.float32)
    wcat = np.concatenate([W, W @ Ablk_s, W @ Ablk_d], axis=1)
    brep = np.broadcast_to(np.asarray(b, np.float32), (P, F)).copy()
    return [
        dict(xt=xt, wcat=wcat, brep=brep,
             gsrci=plan["gsrci"][c], gdlo=plan["gdlo"][c],
             gdhi=plan["gdhi"][c], gloc=plan["gloc"][c])
        for c in range(plan["n_cores"])
    ]


def _build_layer_kernel(plan, edge_dt=EDGE_DT):
    NT, NCH, TOTCH = plan["NT"], plan["NCH"], plan["TOTCH"]
    K, Klo, CO = plan["K"], plan["Klo"], plan["CO"]
    L, NROW, NLO = plan["L"], plan["NROW"], plan["NLO"]
    NCHH = NCH // 2                 # node chunks per half

    nc = bacc.Bacc()
    xt = nc.dram_tensor("xt", [NCH, F, P], f32, kind="ExternalInput")
    wcat = nc.dram_tensor("wcat", [F, FW], f32, kind="ExternalInput")
    brep = nc.dram_tensor("brep", [P, F], f32, kind="ExternalInput")
    gsrci = nc.dram_tensor("gsrci", [P, 8 * TOTCH], i16, kind="ExternalInput")
    gdlo = nc.dram_tensor("gdlo", [P, 8 * TOTCH], i16, kind="ExternalInput")
    gdhi = nc.dram_tensor("gdhi", [P, 8 * TOTCH], i16, kind="ExternalInput")
    gloc = nc.dram_tensor("gloc", [P, TOTCH], f32, kind="ExternalInput")
    out = nc.dram_tensor("out", [NT * P, F], f32, kind="ExternalOutput")

    def rowbase(g):
        """first table row of node chunk g (zero chunk ends each half)"""
        return g * P if g < NCHH else g * P + P

    htab = nc.dram_tensor("htab", [NROW, HROW], f32)
    adtab = nc.dram_tensor("adtab", [NROW, AROW], f32)

    # Phase 1 (its own TileContext: the exit drain+barrier guarantees every
    # htab/adtab byte is in DRAM before any edge-phase gather issues).
    with TileContext(nc) as tc:
        with (
            tc.tile_pool(name="const", bufs=1) as cpool,
            tc.tile_pool(name="nodein", bufs=4) as npool,
            tc.tile_pool(name="nodeout", bufs=4) as hpool,
            tc.tile_pool(name="npsum", bufs=4, space="PSUM") as npsum,
        ):
            wcat_sb = cpool.tile([F, FW], f32)
            nc.sync.dma_start(wcat_sb[:, :], wcat[:, :])

            # zero rows at the end of each half (for wrong-half aD reads)
            zt = cpool.tile([P, HROW], f32)
            nc.vector.memset(zt[:, :], 0.0)
            for zr in (NLO, NROW - P):
                nc.sync.dma_start(htab[zr:zr + P, :], zt[:, :])
                nc.sync.dma_start(adtab[zr:zr + P, :], zt[:, 0:AROW])

            # ---- node phase (batches of NB chunks per DMA) ----
            NB = 4
            assert NCHH % NB == 0
            node_batches = [] if "node" in _SKIP else [
                (b, min(NB, NCHH - (b % NCHH)))
                for b in list(range(0, NCHH, NB)) + list(range(NCHH, NCH, NB))
            ]
            for b, nb in node_batches:
                rb = rowbase(b)
                xcb = npool.tile([F, NB, P], f32, tag="xc")
                nc.sync.dma_start(
                    xcb[:, 0:nb, :],
                    xt[b:b + nb, :, :].rearrange("n f p -> f n p"))
                hcb = hpool.tile([P, NB, FW], f32, tag="hc")
                for k in range(nb):
                    ps = npsum.tile([P, FW], f32, tag="nps")
                    nc.tensor.matmul(ps[:, :], lhsT=xcb[:, k, :],
                                     rhs=wcat_sb[:, :], start=True, stop=True)
                    nc.scalar.copy(hcb[:, k, :], ps[:, :])
                nc.sync.dma_start(
                    htab[rb:rb + nb * P, 0:FA].rearrange("(n p) w -> p n w", p=P),
                    hcb[:, 0:nb, 0:FA])
                nc.sync.dma_start(
                    adtab[rb:rb + nb * P, 0:H].rearrange("(n p) w -> p n w", p=P),
                    hcb[:, 0:nb, FA:FW])

    # Phase 2: edge phase in a second TileContext.
    with TileContext(nc) as tc:
        with (
            tc.tile_pool(name="econst", bufs=1) as cpool,
            tc.tile_pool(name="eidx", bufs=2) as xpool,
            tc.tile_pool(name="egather", bufs=2) as gpool,
            tc.tile_pool(name="eind", bufs=2) as ipool,
            tc.tile_pool(name="ealpha", bufs=3) as apool,
            tc.tile_pool(name="emsg", bufs=2) as mpool,
            tc.tile_pool(name="epsum", bufs=2, space="PSUM") as epsum,
            tc.tile_pool(name="eout", bufs=3) as opool,
        ):
            iota_i = cpool.tile([P, P], i32)
            nc.gpsimd.iota(iota_i[:, :], pattern=[[1, P]], base=0,
                           channel_multiplier=0)
            iota_f = cpool.tile([P, P], f32)
            nc.vector.tensor_copy(iota_f[:, :], iota_i[:, :])
            brep_sb = cpool.tile([P, F], f32)
            nc.sync.dma_start(brep_sb[:, :], brep[:, :])

            for t in range(NT):
                if "edge" in _SKIP:
                    on3 = opool.tile([P, F], f32, tag="on3")
                    nc.vector.memset(on3[:, :], 0.0)
                    nc.sync.dma_start(out[t * P:(t + 1) * P, :], on3[:, :])
                    continue
                Kt, Kl = K[t], Klo[t]
                Kh = Kt - Kl
                co = CO[t]
                src_sb = xpool.tile([P, 8 * Kt], i16, tag="src")
                dlo_sb = xpool.tile([P, 8 * Kt], i16, tag="dlo")
                dhi_sb = xpool.tile([P, 8 * Kt], i16, tag="dhi")
                loc_sb = xpool.tile([P, Kt], f32, tag="loc")
                nc.sync.dma_start(src_sb[:, :], gsrci[:, 8 * co:8 * (co + Kt)])
                nc.sync.dma_start(dlo_sb[:, :], gdlo[:, 8 * co:8 * (co + Kt)])
                nc.sync.dma_start(dhi_sb[:, :], gdhi[:, 8 * co:8 * (co + Kt)])
                nc.sync.dma_start(loc_sb[:, :], gloc[:, co:co + Kt])

                hsa = gpool.tile([P, Kt, HROW], f32, tag="hsa")
                if "hgather" not in _SKIP:
                    nc.gpsimd.dma_gather(
                        out_ap=hsa[:, 0:Kl, :], in_ap=htab[0:L, :],
                        idxs_ap=src_sb[:, 0:8 * Kl],
                        num_idxs=Kl * P, num_idxs_reg=Kl * P, elem_size=HROW,
                        single_packet=False)
                    if Kh > 0:
                        nc.gpsimd.dma_gather(
                            out_ap=hsa[:, Kl:Kt, :], in_ap=htab[L:NROW, :],
                            idxs_ap=src_sb[:, 8 * Kl:8 * Kt],
                            num_idxs=Kh * P, num_idxs_reg=Kh * P, elem_size=HROW,
                            single_packet=False)

                adl = gpool.tile([P, Kt, AROW], f32, tag="adl")
                adh = gpool.tile([P, Kt, AROW], f32, tag="adh")
                if "adgather" in _SKIP:
                    nc.vector.memset(adl[:, :, :], 0.0)
                    nc.vector.memset(adh[:, :, :], 0.0)
                if "hgather" in _SKIP:
                    nc.vector.memset(hsa[:, :, :], 0.0)
                if "adgather" not in _SKIP:
                    nc.gpsimd.dma_gather(
                        out_ap=adl[:, :, :], in_ap=adtab[0:L, :],
                        idxs_ap=dlo_sb[:, :],
                        num_idxs=Kt * P, num_idxs_reg=Kt * P, elem_size=AROW,
                        single_packet=False)
                    nc.gpsimd.dma_gather(
                        out_ap=adh[:, :, :], in_ap=adtab[L:NROW, :],
                        idxs_ap=dhi_sb[:, :],
                        num_idxs=Kt * P, num_idxs_reg=Kt * P, elem_size=AROW,
                        single_packet=False)

                ind = ipool.tile([P, Kt, P], edge_dt, tag="ind")
                for k in range(Kt):
                    nc.vector.tensor_scalar(
                        out=ind[:, k, :], in0=iota_f[:, :],
                        scalar1=loc_sb[:, k:k + 1], scalar2=None,
                        op0=mybir.AluOpType.is_equal,
                    )

                adsum = apool.tile([P, Kt, H], f32, tag="adsum")
                nc.vector.tensor_tensor(
                    out=adsum[:, :, :], in0=adl[:, :, 0:H], in1=adh[:, :, 0:H],
                    op=mybir.AluOpType.add)
                alpha = apool.tile([P, Kt, H], f32, tag="alpha")
                nc.vector.tensor_tensor(
                    out=alpha[:, :, :], in0=hsa[:, :, F:FA],
                    in1=adsum[:, :, :], op=mybir.AluOpType.add)
                lrl = apool.tile([P, Kt, H], f32, tag="lrl")
                nc.vector.scalar_tensor_tensor(
                    out=lrl[:, :, :], in0=alpha[:, :, :], scalar=0.2,
                    in1=alpha[:, :, :],
                    op0=mybir.AluOpType.mult, op1=mybir.AluOpType.max)
                ex = apool.tile([P, Kt, H], f32, tag="ex")
                nc.scalar.activation(ex[:, :, :], lrl[:, :, :],
                                     mybir.ActivationFunctionType.Exp)

                msgex = mpool.tile([P, Kt, FA], edge_dt, tag="msgex")
                nc.vector.tensor_tensor(
                    out=msgex[:, :, 0:F].rearrange("p k (h c) -> p k h c", h=H),
                    in0=hsa[:, :, 0:F].rearrange("p k (h c) -> p k h c", h=H),
                    in1=ex[:, :, :].to_broadcast([P, Kt, H, C]),
                    op=mybir.AluOpType.mult,
                )
                nc.vector.tensor_copy(msgex[:, :, F:FA], ex[:, :, :])

                pso = epsum.tile([P, FA], f32, tag="pso")
                for k in range(Kt):
                    nc.tensor.matmul(pso[:, :], lhsT=ind[:, k, :],
                                     rhs=msgex[:, k, :],
                                     start=(k == 0), stop=(k == Kt - 1))

                den = opool.tile([P, H], f32, tag="den")
                nc.vector.tensor_scalar(out=den[:, :], in0=pso[:, F:FA],
                                        scalar1=1e-16, scalar2=None,
                                        op0=mybir.AluOpType.add)
                rec = opool.tile([P, H], f32, tag="rec")
                nc.vector.reciprocal(rec[:, :], den[:, :])
                on = opool.tile([P, F], f32, tag="on")
                nc.vector.tensor_tensor(
                    out=on[:, :].rearrange("p (h c) -> p h c", h=H),
                    in0=pso[:, 0:F].rearrange("p (h c) -> p h c", h=H),
                    in1=rec[:, :].to_broadcast([P, H, C]),
                    op=mybir.AluOpType.mult,
                )
                on2 = opool.tile([P, F], f32, tag="on2")
                nc.vector.tensor_tensor(out=on2[:, :], in0=on[:, :],
                                        in1=brep_sb[:, :],
                                        op=mybir.AluOpType.add)
                on3 = opool.tile([P, F], f32, tag="on3")
                nc.vector.tensor_scalar(out=on3[:, :], in0=on2[:, :],
                                        scalar1=0.0, scalar2=None,
                                        op0=mybir.AluOpType.max)
                nc.sync.dma_start(out[t * P:(t + 1) * P, :], on3[:, :])

    nc.finalize()
    return nc


_KERNEL_CACHE = {}


def _get_kernel(plan):
    key = (tuple(plan["K"]), tuple(plan["Klo"]), plan["npad"], str(EDGE_DT))
    if key not in _KERNEL_CACHE:
        _KERNEL_CACHE[key] = _build_layer_kernel(plan)
    return _KERNEL_CACHE[key]


def _run_layer(nc, maps, trace=False):
    last = None
    for attempt in range(3):
        try:
            res = run_bass_kernel_spmd(nc, maps, list(range(len(maps))),
                                       trace=trace)
            outs = [r["out"] for r in res.results]
            return np.concatenate(outs, axis=0), res
        except Exception as e:  # transient NRT_EXEC_UNIT_UNRECOVERABLE etc.
            last = e
            import time as _time
            _time.sleep(2.0 * (attempt + 1))
    raise last


def kernel(x, edge_index, W1, a_src1, a_dst1, b1, W2, a_src2, a_dst2, b2,
           _trace=False, _collect=None):
    x = np.asarray(x, dtype=np.float32)
    edge_index = np.asarray(edge_index)
    assert x.shape == (N_NODES, F), x.shape
    assert edge_index.shape == (2, N_EDGES), edge_index.shape

    loops = np.arange(N_NODES, dtype=np.int64)
    src = np.concatenate([edge_index[0].astype(np.int64), loops])
    dst = np.concatenate([edge_index[1].astype(np.int64), loops])

    plan = _make_plan(src, dst, N_NODES, N_CORES)
    nc = _get_kernel(plan)

    maps1 = _layer_inputs(plan, x, np.asarray(W1), np.asarray(a_src1),
                          np.asarray(a_dst1), np.asarray(b1))
    o1, res1 = _run_layer(nc, maps1, trace=_trace)

    maps2 = _layer_inputs(plan, o1[: plan["npad"]], np.asarray(W2),
                          np.asarray(a_src2), np.asarray(a_dst2),
                          np.asarray(b2))
    o2, res2 = _run_layer(nc, maps2, trace=_trace)

    if _collect is not None:
        _collect.extend([res1, res2])
    return o2[:N_NODES].astype(np.float32)


# revision 15
# speedup vs baseline: 2.3893x; 2.3893x over previous
"""Self-contained 2-layer GAT kernel for Trainium2, 8-core SPMD.

Strategy (per the sharding hint): edges are sharded by destination node
across the 8 cores (each core owns a contiguous dst slice; edges sorted by
dst on host). Layer weights are replicated. The node-parallel dense work
(x@W plus the per-node attention halves) is replicated on every core so the
edge phase needs no cross-core halo exchange; the layer-1 -> layer-2
activation reshuffle is pure data movement done on host between the two
launches (all math stays on device).

Per layer, per core:
  node phase:  [h | aS | aD] = xT_chunk.T @ [W | W@Asrc | W@Adst]  (PE)
               -> htab (192-float rows: h|aS|pad) and adtab (64-float rows:
               aD|pad) in DRAM
  edge phase (per 128-dst tile, edges padded to 128-chunks):
      dma_gather h|aS rows by src, aD rows by dst    (int16 indices, so
          tables are split in halves; each tile's edges are bucketed lo/hi
          by src on host; dst reads hit one half per core, the other half's
          gather is pointed at a dedicated zero row)
      alpha = lrelu(aS+aDlo+aDhi); ex = exp(alpha)             (DVE+ACT)
      Ind[e,d] = (iota == dstloc[e]) one-hot                   (DVE)
      PSUM accum over chunks: [out|denom] += Ind.T @ [ex*h | ex]   (PE)
      epilogue: relu(out/denom + b) -> output rows.
"""

import sys
import numpy as np

sys.path.insert(0, "/opt/trn_rl_repo")

import concourse.bacc as bacc
import concourse.mybir as mybir
from concourse.bass_utils import run_bass_kernel_spmd
from concourse.tile import TileContext

f32 = mybir.dt.float32
bf16 = mybir.dt.bfloat16
i16 = mybir.dt.int16
i32 = mybir.dt.int32

P = 128
H = 4
C = 32
F = 128          # feature width (= H*C)
FA = F + H       # h | aS used columns
FW = F + 2 * H   # node-phase matmul width: h | aS | aD
HROW = 192       # htab row floats (768B, 256B multiple for dma_gather)
AROW = 64        # adtab row floats (256B)

N_CORES = 8
N_NODES = 50000
N_EDGES = 800000

EDGE_DT = f32    # dtype for the edge-phase matmul operands

import os
_SKIP = set(os.environ.get("GAT_SKIP", "").split(","))  # perf-bisect flags


def _make_plan(src, dst, N, n_cores):
    npad = ((N + P * n_cores - 1) // (P * n_cores)) * (P * n_cores)
    npc = npad // n_cores
    NT = npc // P
    NCH = npad // P
    NLO = npad // 2                # nodes in the lo half
    L = NLO + P                    # half-table rows (incl. one zero chunk)
    NROW = 2 * L                   # total table rows
    assert L <= 32767 and NLO % P == 0
    ZLO = NLO                      # half-local zero-row id (both halves)

    def rowid_half(v):
        """node id -> (half, half-local row id)"""
        return (v >= NLO).astype(np.int64), np.where(v >= NLO, v - NLO, v)

    tile_of = dst // P
    src_hi = (src >= NLO).astype(np.int64)

    cnt = np.zeros((NCH, 2), np.int64)
    np.add.at(cnt, (tile_of, src_hi), 1)
    cnt_ct = cnt.reshape(n_cores, NT, 2)
    Klo = np.maximum(1, np.ceil(cnt_ct[:, :, 0].max(axis=0) / P).astype(np.int64))
    Khi = np.ceil(cnt_ct[:, :, 1].max(axis=0) / P).astype(np.int64)
    K = Klo + Khi
    TOTCH = int(K.sum())
    CO = np.concatenate([[0], np.cumsum(K)])[:-1]

    okey = tile_of * 2 + src_hi
    order = np.argsort(okey, kind="stable")
    s_src = src[order]
    s_dst = dst[order]
    s_key = okey[order]
    starts = np.searchsorted(s_key, np.arange(NCH * 2))
    ends = np.searchsorted(s_key, np.arange(NCH * 2) + 1)

    gsrci = np.zeros((n_cores, 16, 8 * TOTCH), np.int16)
    gdlo = np.full((n_cores, 16, 8 * TOTCH), ZLO, np.int16)
    gdhi = np.full((n_cores, 16, 8 * TOTCH), ZLO, np.int16)
    gloc = np.full((n_cores, P, TOTCH), float(P), np.float32)

    for c in range(n_cores):
        for t in range(NT):
            g = c * NT + t
            co8 = 8 * CO[t]
            for half, koff, nk in ((0, 0, Klo[t]), (1, Klo[t], K[t] - Klo[t])):
                if nk == 0:
                    continue
                e0, e1 = starts[2 * g + half], ends[2 * g + half]
                n = e1 - e0
                npadn = int(nk) * P
                sv = np.zeros(npadn, np.int64)
                lv = np.full(npadn, P, np.int64)
                dhalf = np.zeros(npadn, np.int64)
                dloc = np.full(npadn, ZLO, np.int64)
                if n:
                    ev = s_src[e0:e1]
                    sv[:n] = np.where(ev >= NLO, ev - NLO, ev)
                    dh, dr = rowid_half(s_dst[e0:e1])
                    dhalf[:n] = dh
                    dloc[:n] = dr
                    lv[:n] = s_dst[e0:e1] % P
                j = np.arange(npadn)
                cc = co8 + 8 * koff + j // 16
                rr = j % 16
                gsrci[c, rr, cc] = sv
                gdlo[c, rr, cc] = np.where(dhalf == 0, dloc, ZLO)
                gdhi[c, rr, cc] = np.where(dhalf == 1, dloc, ZLO)
                kk = CO[t] + koff + j // P
                pp = j % P
                gloc[c, pp, kk] = lv

    gsrci = np.tile(gsrci, (1, 8, 1))
    gdlo = np.tile(gdlo, (1, 8, 1))
    gdhi = np.tile(gdhi, (1, 8, 1))

    return dict(
        n_cores=n_cores, N=N, npad=npad, npc=npc, NT=NT, NCH=NCH,
        NLO=NLO, L=L, NROW=NROW,
        K=[int(k) for k in K], Klo=[int(k) for k in Klo],
        TOTCH=TOTCH, CO=[int(o) for o in CO],
        gsrci=gsrci, gdlo=gdlo, gdhi=gdhi, gloc=gloc,
    )


def _layer_inputs(plan, x, W, a_src, a_dst, b):
    npad, NCH = plan["npad"], plan["NCH"]
    xp = np.zeros((npad, F), dtype=np.float32)
    xp[: x.shape[0]] = x
    xt = np.ascontiguousarray(xp.reshape(NCH, P, F).transpose(0, 2, 1))
    Ablk_s = np.zeros((F, H), dtype=np.float32)
    Ablk_d = np.zeros((F, H), dtype=np.float32)
    for h in range(H):
        Ablk_s[h * C:(h + 1) * C, h] = a_src[h]
        Ablk_d[h * C:(h + 1) * C, h] = a_dst[h]
    W = np.asarray(W, dtype=np.float32)
    wcat = np.concatenate([W, W @ Ablk_s, W @ Ablk_d], axis=1)
    brep = np.broadcast_to(np.asarray(b, np.float32), (P, F)).copy()
    return [
        dict(xt=xt, wcat=wcat, brep=brep,
             gsrci=plan["gsrci"][c], gdlo=plan["gdlo"][c],
             gdhi=plan["gdhi"][c], gloc=plan["gloc"][c])
        for c in range(plan["n_cores"])
    ]


def _build_layer_kernel(plan, edge_dt=EDGE_DT):
    NT, NCH, TOTCH = plan["NT"], plan["NCH"], plan["TOTCH"]
    K, Klo, CO = plan["K"], plan["Klo"], plan["CO"]
    L, NROW, NLO = plan["L"], plan["NROW"], plan["NLO"]
    NCHH = NCH // 2                 # node chunks per half

    nc = bacc.Bacc()
    xt = nc.dram_tensor("xt", [NCH, F, P], f32, kind="ExternalInput")
    wcat = nc.dram_tensor("wcat", [F, FW], f32, kind="ExternalInput")
    brep = nc.dram_tensor("brep", [P, F], f32, kind="ExternalInput")
    gsrci = nc.dram_tensor("gsrci", [P, 8 * TOTCH], i16, kind="ExternalInput")
    gdlo = nc.dram_tensor("gdlo", [P, 8 * TOTCH], i16, kind="ExternalInput")
    gdhi = nc.dram_tensor("gdhi", [P, 8 * TOTCH], i16, kind="ExternalInput")
    gloc = nc.dram_tensor("gloc", [P, TOTCH], f32, kind="ExternalInput")
    out = nc.dram_tensor("out", [NT * P, F], f32, kind="ExternalOutput")

    def rowbase(g):
        """first table row of node chunk g (zero chunk ends each half)"""
        return g * P if g < NCHH else g * P + P

    htab = nc.dram_tensor("htab", [NROW, HROW], f32)
    adtab = nc.dram_tensor("adtab", [NROW, AROW], f32)

    # Phase 1 (its own TileContext: the exit drain+barrier guarantees every
    # htab/adtab byte is in DRAM before any edge-phase gather issues).
    with TileContext(nc) as tc:
        with (
            tc.tile_pool(name="const", bufs=1) as cpool,
            tc.tile_pool(name="nodein", bufs=4) as npool,
            tc.tile_pool(name="nodeout", bufs=4) as hpool,
            tc.tile_pool(name="npsum", bufs=4, space="PSUM") as npsum,
        ):
            wcat_sb = cpool.tile([F, FW], f32)
            nc.sync.dma_start(wcat_sb[:, :], wcat[:, :])

            # zero rows at the end of each half (for wrong-half aD reads)
            zt = cpool.tile([P, HROW], f32)
            nc.vector.memset(zt[:, :], 0.0)
            for zr in (NLO, NROW - P):
                nc.sync.dma_start(htab[zr:zr + P, :], zt[:, :])
                nc.sync.dma_start(adtab[zr:zr + P, :], zt[:, 0:AROW])

            # ---- node phase (batches of NB chunks per DMA) ----
            NB = 8
            node_batches = [] if "node" in _SKIP else [
                (b, min(NB, NCHH - (b % NCHH)))
                for b in list(range(0, NCHH, NB)) + list(range(NCHH, NCH, NB))
            ]
            for b, nb in node_batches:
                rb = rowbase(b)
                xcb = npool.tile([F, NB, P], f32, tag="xc")
                nc.sync.dma_start(
                    xcb[:, 0:nb, :],
                    xt[b:b + nb, :, :].rearrange("n f p -> f n p"))
                hcb = hpool.tile([P, NB, FW], f32, tag="hc")
                for k in range(nb):
                    ps = npsum.tile([P, FW], f32, tag="nps")
                    nc.tensor.matmul(ps[:, :], lhsT=xcb[:, k, :],
                                     rhs=wcat_sb[:, :], start=True, stop=True)
                    nc.scalar.copy(hcb[:, k, :], ps[:, :])
                nc.sync.dma_start(
                    htab[rb:rb + nb * P, 0:FA].rearrange("(n p) w -> p n w", p=P),
                    hcb[:, 0:nb, 0:FA])
                nc.sync.dma_start(
                    adtab[rb:rb + nb * P, 0:H].rearrange("(n p) w -> p n w", p=P),
                    hcb[:, 0:nb, FA:FW])

    # Phase 2: edge phase in a second TileContext.
    with TileContext(nc) as tc:
        with (
            tc.tile_pool(name="econst", bufs=1) as cpool,
            tc.tile_pool(name="eidx", bufs=2) as xpool,
            tc.tile_pool(name="egather", bufs=3) as gpool,
            tc.tile_pool(name="eind", bufs=3) as ipool,
            tc.tile_pool(name="ealpha", bufs=3) as apool,
            tc.tile_pool(name="emsg", bufs=3) as mpool,
            tc.tile_pool(name="epsum", bufs=4, space="PSUM") as epsum,
            tc.tile_pool(name="eout", bufs=3) as opool,
        ):
            iota_i = cpool.tile([P, P], i32)
            nc.gpsimd.iota(iota_i[:, :], pattern=[[1, P]], base=0,
                           channel_multiplier=0)
            iota_f = cpool.tile([P, P], f32)
            nc.vector.tensor_copy(iota_f[:, :], iota_i[:, :])
            brep_sb = cpool.tile([P, F], f32)
            nc.sync.dma_start(brep_sb[:, :], brep[:, :])

            # whole-layer index arrays resident in SBUF (4 DMAs total)
            srcA = cpool.tile([P, 8 * TOTCH], i16)
            nc.sync.dma_start(srcA[:, :], gsrci[:, :])
            dloA = cpool.tile([P, 8 * TOTCH], i16)
            nc.sync.dma_start(dloA[:, :], gdlo[:, :])
            dhiA = cpool.tile([P, 8 * TOTCH], i16)
            nc.sync.dma_start(dhiA[:, :], gdhi[:, :])
            locA = cpool.tile([P, TOTCH], f32)
            nc.sync.dma_start(locA[:, :], gloc[:, :])

            for t in range(NT):
                if "edge" in _SKIP:
                    on3 = opool.tile([P, F], f32, tag="on3")
                    nc.vector.memset(on3[:, :], 0.0)
                    nc.sync.dma_start(out[t * P:(t + 1) * P, :], on3[:, :])
                    continue
                Kt, Kl = K[t], Klo[t]
                Kh = Kt - Kl
                co = CO[t]
                src_sb = srcA[:, 8 * co:8 * (co + Kt)]
                dlo_sb = dloA[:, 8 * co:8 * (co + Kt)]
                dhi_sb = dhiA[:, 8 * co:8 * (co + Kt)]
                loc_sb = locA[:, co:co + Kt]

                hsa = gpool.tile([P, Kt, HROW], f32, tag="hsa")
                if "hgather" not in _SKIP:
                    nc.gpsimd.dma_gather(
                        out_ap=hsa[:, 0:Kl, :], in_ap=htab[0:L, :],
                        idxs_ap=srcA[:, 8 * co:8 * co + 8 * Kl],
                        num_idxs=Kl * P, num_idxs_reg=Kl * P, elem_size=HROW,
                        single_packet=False)
                    if Kh > 0:
                        nc.gpsimd.dma_gather(
                            out_ap=hsa[:, Kl:Kt, :], in_ap=htab[L:NROW, :],
                            idxs_ap=srcA[:, 8 * (co + Kl):8 * (co + Kt)],
                            num_idxs=Kh * P, num_idxs_reg=Kh * P, elem_size=HROW,
                            single_packet=False)

                adl = gpool.tile([P, Kt, AROW], f32, tag="adl")
                adh = gpool.tile([P, Kt, AROW], f32, tag="adh")
                if "adgather" in _SKIP:
                    nc.vector.memset(adl[:, :, :], 0.0)
                    nc.vector.memset(adh[:, :, :], 0.0)
                if "hgather" in _SKIP:
                    nc.vector.memset(hsa[:, :, :], 0.0)
                if "adgather" not in _SKIP:
                    nc.gpsimd.dma_gather(
                        out_ap=adl[:, :, :], in_ap=adtab[0:L, :],
                        idxs_ap=dloA[:, 8 * co:8 * (co + Kt)],
                        num_idxs=Kt * P, num_idxs_reg=Kt * P, elem_size=AROW,
                        single_packet=False)
                    nc.gpsimd.dma_gather(
                        out_ap=adh[:, :, :], in_ap=adtab[L:NROW, :],
                        idxs_ap=dhiA[:, 8 * co:8 * (co + Kt)],
                        num_idxs=Kt * P, num_idxs_reg=Kt * P, elem_size=AROW,
                        single_packet=False)

                ind = ipool.tile([P, Kt, P], edge_dt, tag="ind")
                for k in range(Kt):
                    nc.vector.tensor_scalar(
                        out=ind[:, k, :], in0=iota_f[:, :],
                        scalar1=locA[:, co + k:co + k + 1], scalar2=None,
                        op0=mybir.AluOpType.is_equal,
                    )

                adsum = apool.tile([P, Kt, H], f32, tag="adsum")
                nc.vector.tensor_tensor(
                    out=adsum[:, :, :], in0=adl[:, :, 0:H], in1=adh[:, :, 0:H],
                    op=mybir.AluOpType.add)
                alpha = apool.tile([P, Kt, H], f32, tag="alpha")
                nc.vector.tensor_tensor(
                    out=alpha[:, :, :], in0=hsa[:, :, F:FA],
                    in1=adsum[:, :, :], op=mybir.AluOpType.add)
                lrl = apool.tile([P, Kt, H], f32, tag="lrl")
                nc.vector.scalar_tensor_tensor(
                    out=lrl[:, :, :], in0=alpha[:, :, :], scalar=0.2,
                    in1=alpha[:, :, :],
                    op0=mybir.AluOpType.mult, op1=mybir.AluOpType.max)
                ex = apool.tile([P, Kt, H], f32, tag="ex")
                nc.scalar.activation(ex[:, :, :], lrl[:, :, :],
                                     mybir.ActivationFunctionType.Exp)

                msgex = mpool.tile([P, Kt, FA], edge_dt, tag="msgex")
                nc.vector.tensor_tensor(
                    out=msgex[:, :, 0:F].rearrange("p k (h c) -> p k h c", h=H),
                    in0=hsa[:, :, 0:F].rearrange("p k (h c) -> p k h c", h=H),
                    in1=ex[:, :, :].to_broadcast([P, Kt, H, C]),
                    op=mybir.AluOpType.mult,
                )
                nc.vector.tensor_copy(msgex[:, :, F:FA], ex[:, :, :])

                pso = epsum.tile([P, FA], f32, tag="pso")
                for k in range(Kt):
                    nc.tensor.matmul(pso[:, :], lhsT=ind[:, k, :],
                                     rhs=msgex[:, k, :],
                                     start=(k == 0), stop=(k == Kt - 1))

                den = opool.tile([P, H], f32, tag="den")
                nc.vector.tensor_scalar(out=den[:, :], in0=pso[:, F:FA],
                                        scalar1=1e-16, scalar2=None,
                                        op0=mybir.AluOpType.add)
                rec = opool.tile([P, H], f32, tag="rec")
                nc.vector.reciprocal(rec[:, :], den[:, :])
                on = opool.tile([P, F], f32, tag="on")
                nc.vector.tensor_tensor(
                    out=on[:, :].rearrange("p (h c) -> p h c", h=H),
                    in0=pso[:, 0:F].rearrange("p (h c) -> p h c", h=H),
                    in1=rec[:, :].to_broadcast([P, H, C]),
                    op=mybir.AluOpType.mult,
                )
                on2 = opool.tile([P, F], f32, tag="on2")
                nc.vector.tensor_tensor(out=on2[:, :], in0=on[:, :],
                                        in1=brep_sb[:, :],
                                        op=mybir.AluOpType.add)
                on3 = opool.tile([P, F], f32, tag="on3")
                nc.vector.tensor_scalar(out=on3[:, :], in0=on2[:, :],
                                        scalar1=0.0, scalar2=None,
                                        op0=mybir.AluOpType.max)
                nc.sync.dma_start(out[t * P:(t + 1) * P, :], on3[:, :])

    nc.finalize()
    return nc


_KERNEL_CACHE = {}


def _get_kernel(plan):
    key = (tuple(plan["K"]), tuple(plan["Klo"]), plan["npad"], str(EDGE_DT))
    if key not in _KERNEL_CACHE:
        _KERNEL_CACHE[key] = _build_layer_kernel(plan)
    return _KERNEL_CACHE[key]


def _run_layer(nc, maps, trace=False):
    last = None
    for attempt in range(3):
        try:
            res = run_bass_kernel_spmd(nc, maps, list(range(len(maps))),
                                       trace=trace)
            outs = [r["out"] for r in res.results]
            return np.concatenate(outs, axis=0), res
        except Exception as e:  # transient NRT_EXEC_UNIT_UNRECOVERABLE etc.
            last = e
            import time as _time
            _time.sleep(2.0 * (attempt + 1))
    raise last


def kernel(x, edge_index, W1, a_src1, a_dst1, b1, W2, a_src2, a_dst2, b2,
           _trace=False, _collect=None):
    x = np.asarray(x, dtype=np.float32)
    edge_index = np.asarray(edge_index)
    assert x.shape == (N_NODES, F), x.shape
    assert edge_index.shape == (2, N_EDGES), edge_index.shape

    loops = np.arange(N_NODES, dtype=np.int64)
    src = np.concatenate([edge_index[0].astype(np.int64), loops])
    dst = np.concatenate([edge_index[1].astype(np.int64), loops])

    plan = _make_plan(src, dst, N_NODES, N_CORES)
    nc = _get_kernel(plan)

    maps1 = _layer_inputs(plan, x, np.asarray(W1), np.asarray(a_src1),
                          np.asarray(a_dst1), np.asarray(b1))
    o1, res1 = _run_layer(nc, maps1, trace=_trace)

    maps2 = _layer_inputs(plan, o1[: plan["npad"]], np.asarray(W2),
                          np.asarray(a_src2), np.asarray(a_dst2),
                          np.asarray(b2))
    o2, res2 = _run_layer(nc, maps2, trace=_trace)

    if _collect is not None:
        _collect.extend([res1, res2])
    return o2[:N_NODES].astype(np.float32)
